# revision 18
# baseline (speedup 1.0000x reference)
"""Trainium2 Bass kernel for KeyChannelwiseMemoryMultiHead.

Math per pixel vector x (256 channels):
  y1 = w_in @ x + b_in; per-head key matmul; softmax over mem dim;
  per-head memory matmul; w_out @ . + b_out.

Host-side exact refactor (fp64 weight folding):
  KW[(n,m), c] = sum_k key_p[n,k,m] w_in[k*8+n, c]   -> stage A: A = KW @ x
  kb[(n,m)]    = sum_k key_p[n,k,m] b_in[k*8+n]      -> folded as exp bias
  WM[o, (n,m)] = sum_d w_out[o, n*64+d] memory[n,m,d]
  E = exp(A + kb);  wsum[n] = sum_m E;  S = E / wsum
  out = WM @ S + b_out

On-chip (per core = one batch, pixel chunks [512 x7, 256, 256]):
  stage A: 2 K-tile bf16 matmuls -> PSUM [128 nm, P pix] (4 nm tiles)
  exp:     ScalarE activation(Exp, bias=kb) PSUM->SBUF
  wsum:    matmul with block-diagonal ones [128,128] (head-indicator)
  recip:   fused DVE op S = E * approx_recip(wsum)  PSUM->SBUF
  stage B: 8 accumulating bf16 matmuls -> PSUM [128 out, P pix]
  bias:    ScalarE identity+bias (o=0) / DVE tensor_scalar_add (o=1), bf16.

Perf structure:
  - The head is HBM-bandwidth-bound (8 cores share the fabric; per-core
    ~125GB/s effective): kw + chunk-0 x are fused into 2 blobs serial on
    the sync queue, chunk 0 runs i-half-major so blob1 alone unblocks 4
    matmuls. Chunk 1's x rides the scalar queue as TWO half DMAs so its
    i=0 matmuls can start as soon as the first half lands.
  - A dummy exp (gated only on the wz memset) heads the scalar stream so
    the lazy ACT_TABLE_LOAD's ~1.3us table fetch runs in the DMA-free
    window before b1's packets start.
  - 16 small warmup matmuls (gated only on the wz memset) start the PE
    p-state ramp at ~6.1us, right when the instruction stream begins
    (PE clock ramps 0.65->2.4GHz over ~3-4.5us of continuous activity;
    idle gaps >~0.5us reset the ramp), so the first real A matmuls and
    the whole first chunk run near full clock.
  - The last two chunks are 256px: the final drain (exp/recip already
    overlapped; B + bias + last output DMA remain) covers half the
    bytes, and the final DMA is split o0/o1 across scalar/sync queues.
  - Tensor-engine issue order per chunk j: A(j) | wsum(j,t0,t1) | B(j-1) |
    wsum(j,t2,t3) -- hides exp/recip latency behind the previous chunk's B.
  - PSUM banks: pa=4, ps=2, po=2x1.
  - Output stored bf16 (host converts to fp32).
"""

import os
import sys

import numpy as np

for _p in ("/opt/trn_rl_repo", "/root/.axon_site/_ro/trn_rl_repo"):
    if os.path.isdir(_p) and _p not in sys.path:
        sys.path.insert(0, _p)

import concourse.bass as bass  # noqa: E402
import concourse.tile as tile  # noqa: E402
from concourse import bacc, bass_utils, mybir  # noqa: E402
from concourse import dve_ops as _dve_ops  # noqa: E402
from concourse.dve_spec import (  # noqa: E402
    AluOp,
    Bin,
    C0,
    C1,
    Spec,
    Src0,
    Src1,
    _has_src1,
    lower,
)
from concourse.dve_uop import DveOpSpec  # noqa: E402

N_CORES = 8
C_IN = 256
NM = 512
C_OUT = 256
NPIX = 64 * 64
FP32 = mybir.dt.float32
BF16 = mybir.dt.bfloat16
_RC0 = -0.23549792
_RC1 = 2.0017324

CHUNKS = [512, 512, 512, 512, 512, 512, 512, 256, 256]
STARTS = [sum(CHUNKS[:i]) for i in range(len(CHUNKS))]
NCH = len(CHUNKS)
assert sum(CHUNKS) == NPIX

_FUSED_OP = None


def _register_fused_divmul():
    """out = in1 * approx_recip(in0): BITWISE_NOT exponent-flip seed +
    one inline Newton pass + multiply by in1 -- single DVE pass."""
    global _FUSED_OP
    if _FUSED_OP is not None:
        return _FUSED_OP
    name = "RECIP1NR_MUL_ANT"
    _not_x = Bin(AluOp.BITWISE_NOT, Src0, Src0)
    _y0 = _not_x * C0
    _y1 = _y0 * (C1 - Src0 * _y0)

    def _ref(in0, in1, c0, c1, c2):
        not_x = (~in0.view(np.int32)).view(np.float32)
        y0 = not_x * c0
        y1 = y0 * (c1 - in0 * y0)
        return y1 * in1

    spec = Spec(body=_y1 * Src1, reference=_ref)
    row = max(_dve_ops._SUB_OPCODE_FOR_NAME.values()) + 1
    assert row < 0x20
    _dve_ops._SUB_OPCODE_FOR_NAME[name] = row
    shas = {}
    for ver in ("v3",):
        s = DveOpSpec(name=name, opcode=row, uops=lower(spec, ver=ver),
                      rd1_en=_has_src1(spec))
        shas[ver] = s.sha(ver)
    op = _dve_ops.DveOp(name, spec, subdim=False, uops_sha=shas)
    _dve_ops.OPS.append(op)
    _dve_ops.CUSTOM_DVE_SPECS[name] = spec
    _FUSED_OP = op
    return op

_CACHED_NC = None


def _build_nc():
    nc = bacc.Bacc(
        "TRN2",
        target_bir_lowering=False,
        debug=False,
        enable_asserts=True,
        num_devices=N_CORES,
    )
    # b1 row p: [kwt[p, :] (512) | x[p, 0:512]]      (k-half 0 weights + x)
    # b2 row p: [kwt[128+p, :] (512) | x[128+p, 0:512]]
    b1_d = nc.dram_tensor("b1", [128, 1024], BF16, kind="ExternalInput")
    b2_d = nc.dram_tensor("b2", [128, 1024], BF16, kind="ExternalInput")
    # wrest cols: [sumw 128 | wmt 4x256]
    wrest_d = nc.dram_tensor("wrest", [128, 1152], BF16, kind="ExternalInput")
    # wf cols: [kb tile0..3 | bout o0 | bout o1]
    wf_d = nc.dram_tensor("wf", [128, 6], FP32, kind="ExternalInput")
    # xd row p: chunks 1..8, per chunk [i=0..1][c] = x[i*128+p, s+c]
    xd_d = nc.dram_tensor("xd", [128, 7 * 1024], BF16, kind="ExternalInput")
    # od row p: per chunk at pixel s width P: cols [2s + o*P + k]
    od_d = nc.dram_tensor("od", [128, 8192], BF16, kind="ExternalOutput")

    Exp = mybir.ActivationFunctionType.Exp
    Ident = mybir.ActivationFunctionType.Identity
    fused = _register_fused_divmul()

    # xd column offset of chunk j (j >= 1); chunk 0's x lives in b1/b2
    xoff = [0] * NCH
    for j in range(1, NCH):
        xoff[j] = 2 * (STARTS[j] - 512)

    with tile.TileContext(nc) as tc:
        with (
            tc.tile_pool(name="wpool", bufs=1) as wpool,
            tc.tile_pool(name="warm", bufs=1) as warm,
            tc.tile_pool(name="xpool", bufs=4) as xpool,
            tc.tile_pool(name="epool", bufs=5) as epool,
            tc.tile_pool(name="spool", bufs=6) as spool,
            tc.tile_pool(name="opool", bufs=3) as opool,
            tc.tile_pool(name="pa", bufs=4, space="PSUM") as pa,
            tc.tile_pool(name="ps", bufs=2, space="PSUM") as ps,
            tc.tile_pool(name="po", bufs=1, space="PSUM") as po,
        ):
            # --- act-table preload: dummy exp (gated only on the wz
            # memset) is the FIRST scalar instruction, so the lazy
            # ACT_TABLE_LOAD runs in the DMA-free window before b1's
            # packets start, and the first real exp is then gated only
            # by its PSUM input.
            wz = warm.tile([128, 256], BF16, name="wz", tag="wz")
            nc.gpsimd.memset(wz[:], 0)
            ez = warm.tile([128, 16], BF16, name="ez", tag="ez")
            nc.scalar.activation(ez[:], wz[:, 0:16], Exp)

            # --- DMAs, latency-ordered. b1+b2 serial on the sync queue
            # (b1 gets full DMA throughput); x1 on the scalar queue in
            # two halves so chunk 1's i=0 matmuls can start early.
            b1 = wpool.tile([128, 1024], BF16, name="b1", tag="b1")
            nc.sync.dma_start(b1[:], b1_d[:, :])
            b2 = wpool.tile([128, 1024], BF16, name="b2", tag="b2")
            nc.sync.dma_start(b2[:], b2_d[:, :])
            wrest = wpool.tile([128, 1152], BF16, name="wrest", tag="wrest")
            nc.sync.dma_start(wrest[:], wrest_d[:, :])
            wf = wpool.tile([128, 6], FP32, name="wf", tag="wf")
            nc.sync.dma_start(wf[:], wf_d[:, :])

            xt = {}  # chunk j -> (tile, col offset)

            def load_x(j, eng=None, halves=False):
                t_ = xpool.tile([128, 1024], BF16, name=f"x{j}", tag="x")
                q = eng or nc.sync
                w = 2 * CHUNKS[j] + (2 * CHUNKS[j + 1] if j == NCH - 2 else 0)
                if halves:
                    q.dma_start(t_[:, 0 : w // 2],
                                xd_d[:, xoff[j] : xoff[j] + w // 2])
                    q.dma_start(t_[:, w // 2 : w],
                                xd_d[:, xoff[j] + w // 2 : xoff[j] + w])
                else:
                    q.dma_start(t_[:, 0:w], xd_d[:, xoff[j] : xoff[j] + w])
                xt[j] = (t_, 0)
                if j == NCH - 2:
                    xt[j + 1] = (t_, 2 * CHUNKS[j])

            load_x(1, nc.scalar, halves=True)
            load_x(2)
            load_x(3)

            # --- PE warmup: ramp the p-state from the very start of the
            # instruction stream; gated only on the wz memset.
            for w in range(16):
                wp = pa.tile([128, 512], FP32, name=f"warm{w}", tag="pa")
                fz = 128 if w < 10 else 256
                nc.tensor.matmul(
                    wp[:, 0:fz], wz[:, 0:128], wz[:, 0:fz],
                    start=True, stop=True,
                )

            kw = [b1[:, 0:512], b2[:, 0:512]]
            sumw = wrest[:, 0:128]

            prev = None  # (s_tiles, po_t, j_prev)

            def issue_B(state):
                s_tiles, po_t, jp = state
                P = CHUNKS[jp]
                for t in range(4):
                    for o in range(2):
                        nc.tensor.matmul(
                            po_t[o][:, 0:P],
                            wrest[:, 128 + t * 256 + o * 128 : 128 + t * 256 + (o + 1) * 128],
                            s_tiles[t][:, 0:P],
                            start=(t == 0),
                            stop=(t == 3),
                        )

            def issue_tail(state, last=False):
                s_tiles, po_t, jp = state
                P = CHUNKS[jp]
                s0 = 2 * STARTS[jp]
                o_sb = opool.tile([128, 1024], BF16, name=f"o_{jp}", tag="o")
                nc.scalar.activation(
                    o_sb[:, 0:P], po_t[0][:, 0:P], Ident, bias=wf[:, 4:5]
                )
                if last:
                    # o0 half goes out via the scalar engine's own HWDGE
                    # queue, in parallel with sync handling the o1 half.
                    nc.scalar.dma_start(
                        od_d[:, s0 : s0 + P], o_sb[:, 0:P]
                    )
                nc.vector.tensor_scalar_add(
                    o_sb[:, P : 2 * P], po_t[1][:, 0:P], wf[:, 5:6]
                )
                if last:
                    nc.sync.dma_start(
                        od_d[:, s0 + P : s0 + 2 * P], o_sb[:, P : 2 * P]
                    )
                else:
                    nc.sync.dma_start(
                        od_d[:, s0 : s0 + 2 * P], o_sb[:, 0 : 2 * P]
                    )

            for j in range(NCH):
                P = CHUNKS[j]
                if j == 0:
                    xi = [b1[:, 512:1024], b2[:, 512:1024]]
                else:
                    xc, off = xt.pop(j)
                    xi = [xc[:, off : off + P], xc[:, off + P : off + 2 * P]]

                # ---- stage A ----
                a_ps = [
                    pa.tile([128, 512], FP32, name=f"pa_{j}_{t}", tag="pa")
                    for t in range(4)
                ]
                if j <= 1:
                    # i-half-major: the 4 i=0 matmuls only need the first
                    # half of the chunk's input (blob1 / x1's first half).
                    for i in range(2):
                        for t in range(4):
                            nc.tensor.matmul(
                                a_ps[t][:, 0:P],
                                kw[i][:, t * 128 : (t + 1) * 128],
                                xi[i],
                                start=(i == 0),
                                stop=(i == 1),
                            )
                else:
                    for t in range(4):
                        for i in range(2):
                            nc.tensor.matmul(
                                a_ps[t][:, 0:P],
                                kw[i][:, t * 128 : (t + 1) * 128],
                                xi[i],
                                start=(i == 0),
                                stop=(i == 1),
                            )

                e_sb = []
                for t in range(4):
                    e_ = epool.tile([128, 512], BF16, name=f"e_{j}_{t}", tag="e")
                    nc.scalar.activation(
                        e_[:, 0:P], a_ps[t][:, 0:P], Exp, bias=wf[:, t : t + 1]
                    )
                    e_sb.append(e_)

                s_tiles = []
                for t in range(2):
                    p_ = ps.tile([128, 512], FP32, name=f"ps_{j}_{t}", tag="ps")
                    nc.tensor.matmul(
                        p_[:, 0:P], sumw, e_sb[t][:, 0:P], start=True, stop=True
                    )
                    s_ = spool.tile([128, 512], BF16, name=f"s_{j}_{t}", tag="s")
                    nc.vector._custom_dve(
                        fused, out=s_[:, 0:P], in0=p_[:, 0:P], in1=e_sb[t][:, 0:P],
                        s0=_RC0, s1=_RC1,
                    )
                    s_tiles.append(s_)

                if prev is not None:
                    issue_B(prev)

                for t in range(2, 4):
                    p_ = ps.tile([128, 512], FP32, name=f"ps_{j}_{t}", tag="ps")
                    nc.tensor.matmul(
                        p_[:, 0:P], sumw, e_sb[t][:, 0:P], start=True, stop=True
                    )
                    s_ = spool.tile([128, 512], BF16, name=f"s_{j}_{t}", tag="s")
                    nc.vector._custom_dve(
                        fused, out=s_[:, 0:P], in0=p_[:, 0:P], in1=e_sb[t][:, 0:P],
                        s0=_RC0, s1=_RC1,
                    )
                    s_tiles.append(s_)

                if prev is not None:
                    issue_tail(prev)

                po_t = [
                    po.tile([128, 512], FP32, name=f"po{o}_{j}", tag=f"po{o}")
                    for o in range(2)
                ]
                prev = (s_tiles, po_t, j)

                if j + 4 <= NCH - 2:
                    load_x(j + 4)

            issue_B(prev)
            issue_tail(prev, last=True)

    nc.compile()
    return nc


def _fold_weights(key_p, memory, w_in, b_in, w_out, b_out):
    import ml_dtypes

    key_p = np.asarray(key_p, np.float64)
    memory = np.asarray(memory, np.float64)
    w_in = np.asarray(w_in, np.float64)
    b_in = np.asarray(b_in, np.float64)
    w_out = np.asarray(w_out, np.float64)
    b_out = np.asarray(b_out, np.float64)

    w_in_r = w_in.reshape(64, 8, C_IN)  # [k, n, c]
    kw = np.einsum("nkm,knc->nmc", key_p, w_in_r)  # [n, m, c]
    kwt = kw.reshape(NM, C_IN).T.astype(ml_dtypes.bfloat16)  # [c, nm]

    kb = np.einsum("nkm,kn->nm", key_p, b_in.reshape(64, 8)).reshape(NM)

    w_out_r = w_out.reshape(C_OUT, 8, 64)  # [o, n, d]
    wm = np.einsum("ond,nmd->onm", w_out_r, memory)  # [o, n, m]
    wmt = wm.reshape(C_OUT, NM).T  # [nm, o]

    wrest = np.zeros((128, 1152), ml_dtypes.bfloat16)
    blk = np.zeros((128, 128))
    blk[:64, :64] = 1.0
    blk[64:, 64:] = 1.0
    wrest[:, 0:128] = blk.astype(ml_dtypes.bfloat16)
    for t in range(4):
        wrest[:, 128 + t * 256 : 128 + (t + 1) * 256] = (
            wmt[t * 128 : (t + 1) * 128, :].astype(ml_dtypes.bfloat16)
        )

    wf = np.zeros((128, 6), np.float32)
    wf[:, 0:4] = kb.reshape(4, 128).T
    wf[:, 4:6] = b_out.reshape(2, 128).T
    return kwt, np.ascontiguousarray(wrest), wf


import ml_dtypes as _mld

_ml_bf16 = _mld.bfloat16


def kernel_with_results(trace=False, tmpdir=None, **inputs):
    global _CACHED_NC
    x = np.asarray(inputs["x"], np.float32)  # [8, 256, 64, 64]
    kwt, wrest, wf = _fold_weights(
        inputs["key_p"],
        inputs["memory"],
        inputs["w_in"],
        inputs["b_in"],
        inputs["w_out"],
        inputs["b_out"],
    )
    if _CACHED_NC is None:
        _CACHED_NC = _build_nc()
    nc = _CACHED_NC

    in_maps = []
    for b in range(N_CORES):
        xb = x[b].reshape(C_IN, NPIX).astype(_ml_bf16)  # [c, pix]
        b1 = np.concatenate([kwt[0:128], xb[0:128, 0:512]], axis=1)
        b2 = np.concatenate([kwt[128:256], xb[128:256, 0:512]], axis=1)
        # xd: chunks 1..8, per chunk [i0 block | i1 block]
        cols = []
        for j in range(1, NCH):
            s, P = STARTS[j], CHUNKS[j]
            cols.append(xb[0:128, s : s + P])
            cols.append(xb[128:256, s : s + P])
        xd = np.concatenate(cols, axis=1)
        in_maps.append(
            {
                "b1": np.ascontiguousarray(b1),
                "b2": np.ascontiguousarray(b2),
                "wrest": wrest,
                "wf": wf,
                "xd": np.ascontiguousarray(xd),
            }
        )

    res = bass_utils.run_bass_kernel_spmd(
        nc, in_maps, core_ids=list(range(N_CORES)), trace=trace, tmpdir=tmpdir
    )
    outs = []
    for b in range(N_CORES):
        od = np.asarray(res.results[b]["od"]).astype(np.float32)
        out = np.empty((C_OUT, NPIX), np.float32)
        for j in range(NCH):
            s, P = STARTS[j], CHUNKS[j]
            blk = od[:, 2 * s : 2 * s + 2 * P]
            out[0:128, s : s + P] = blk[:, 0:P]
            out[128:256, s : s + P] = blk[:, P : 2 * P]
        outs.append(out.reshape(C_OUT, 64, 64))
    out = np.stack(outs)
    return out, res


def kernel(**inputs):
    out, _ = kernel_with_results(trace=False, **inputs)
    return out


# revision 19
# speedup vs baseline: 1.0369x; 1.0369x over previous
"""Trainium2 Bass kernel for KeyChannelwiseMemoryMultiHead.

Math per pixel vector x (256 channels):
  y1 = w_in @ x + b_in; per-head key matmul; softmax over mem dim;
  per-head memory matmul; w_out @ . + b_out.

Host-side exact refactor (fp64 weight folding):
  KW[(n,m), c] = sum_k key_p[n,k,m] w_in[k*8+n, c]   -> stage A: A = KW @ x
  kb[(n,m)]    = sum_k key_p[n,k,m] b_in[k*8+n]      -> folded as exp bias
  WM[o, (n,m)] = sum_d w_out[o, n*64+d] memory[n,m,d]
  E = exp(A + kb);  wsum[n] = sum_m E;  S = E / wsum
  out = WM @ S + b_out

On-chip (per core = one batch, pixel chunks [512 x7, 256, 256]):
  stage A: 2 K-tile bf16 matmuls -> PSUM [128 nm, P pix] (4 nm tiles)
  exp:     ScalarE activation(Exp, bias=kb) PSUM->SBUF
  wsum:    matmul with block-diagonal ones [128,128] (head-indicator)
  recip:   fused DVE op S = E * approx_recip(wsum)  PSUM->SBUF
  stage B: 8 accumulating bf16 matmuls -> PSUM [128 out, P pix]
  bias:    ScalarE identity+bias (o=0) / DVE tensor_scalar_add (o=1), bf16.

Perf structure:
  - The head is HBM-bandwidth-bound (8 cores share the fabric; per-core
    ~125GB/s effective): kw + chunk-0 x are fused into 2 blobs serial on
    the sync queue, chunk 0 runs i-half-major so blob1 alone unblocks 4
    matmuls. Chunk 1's x rides the scalar queue as TWO half DMAs so its
    i=0 matmuls can start as soon as the first half lands.
  - A dummy exp (gated only on the wz memset) heads the scalar stream so
    the lazy ACT_TABLE_LOAD's ~1.3us table fetch runs in the DMA-free
    window before b1's packets start.
  - 16 small warmup matmuls (gated only on the wz memset) start the PE
    p-state ramp at ~6.1us, right when the instruction stream begins
    (PE clock ramps 0.65->2.4GHz over ~3-4.5us of continuous activity;
    idle gaps >~0.5us reset the ramp), so the first real A matmuls and
    the whole first chunk run near full clock.
  - The last two chunks are 256px: the final drain (exp/recip already
    overlapped; B + bias + last output DMA remain) covers half the
    bytes, and the final DMA is split o0/o1 across scalar/sync queues.
  - Tensor-engine issue order per chunk j: A(j) | wsum(j,t0,t1) | B(j-1) |
    wsum(j,t2,t3) -- hides exp/recip latency behind the previous chunk's B.
  - PSUM banks: pa=4, ps=2, po=2x1.
  - Output stored bf16 (host converts to fp32).
"""

import os
import sys

import numpy as np

for _p in ("/opt/trn_rl_repo", "/root/.axon_site/_ro/trn_rl_repo"):
    if os.path.isdir(_p) and _p not in sys.path:
        sys.path.insert(0, _p)

import concourse.bass as bass  # noqa: E402
import concourse.tile as tile  # noqa: E402
from concourse import bacc, bass_utils, mybir  # noqa: E402
from concourse import dve_ops as _dve_ops  # noqa: E402
from concourse.dve_spec import (  # noqa: E402
    AluOp,
    Bin,
    C0,
    C1,
    Spec,
    Src0,
    Src1,
    _has_src1,
    lower,
)
from concourse.dve_uop import DveOpSpec  # noqa: E402

N_CORES = 8
C_IN = 256
NM = 512
C_OUT = 256
NPIX = 64 * 64
FP32 = mybir.dt.float32
BF16 = mybir.dt.bfloat16
_RC0 = -0.23549792
_RC1 = 2.0017324

CHUNKS = [512, 512, 512, 512, 512, 512, 512, 256, 256]
STARTS = [sum(CHUNKS[:i]) for i in range(len(CHUNKS))]
NCH = len(CHUNKS)
assert sum(CHUNKS) == NPIX

_FUSED_OP = None


def _register_fused_divmul():
    """out = in1 * approx_recip(in0): BITWISE_NOT exponent-flip seed +
    one inline Newton pass + multiply by in1 -- single DVE pass."""
    global _FUSED_OP
    if _FUSED_OP is not None:
        return _FUSED_OP
    name = "RECIP1NR_MUL_ANT"
    _not_x = Bin(AluOp.BITWISE_NOT, Src0, Src0)
    _y0 = _not_x * C0
    _y1 = _y0 * (C1 - Src0 * _y0)

    def _ref(in0, in1, c0, c1, c2):
        not_x = (~in0.view(np.int32)).view(np.float32)
        y0 = not_x * c0
        y1 = y0 * (c1 - in0 * y0)
        return y1 * in1

    spec = Spec(body=_y1 * Src1, reference=_ref)
    row = max(_dve_ops._SUB_OPCODE_FOR_NAME.values()) + 1
    assert row < 0x20
    _dve_ops._SUB_OPCODE_FOR_NAME[name] = row
    shas = {}
    for ver in ("v3",):
        s = DveOpSpec(name=name, opcode=row, uops=lower(spec, ver=ver),
                      rd1_en=_has_src1(spec))
        shas[ver] = s.sha(ver)
    op = _dve_ops.DveOp(name, spec, subdim=False, uops_sha=shas)
    _dve_ops.OPS.append(op)
    _dve_ops.CUSTOM_DVE_SPECS[name] = spec
    _FUSED_OP = op
    return op

_CACHED_NC = None


def _build_nc():
    nc = bacc.Bacc(
        "TRN2",
        target_bir_lowering=False,
        debug=False,
        enable_asserts=True,
        num_devices=N_CORES,
    )
    # b1 row p: [kwt[p, :] (512) | x[p, 0:512]]      (k-half 0 weights + x)
    # b2 row p: [kwt[128+p, :] (512) | x[128+p, 0:512]]
    b1_d = nc.dram_tensor("b1", [128, 1024], BF16, kind="ExternalInput")
    b2_d = nc.dram_tensor("b2", [128, 1024], BF16, kind="ExternalInput")
    # wrest cols: [sumw 128 | wmt 4x256]
    wrest_d = nc.dram_tensor("wrest", [128, 1152], BF16, kind="ExternalInput")
    # wf cols: [kb tile0..3 | bout o0 | bout o1]
    wf_d = nc.dram_tensor("wf", [128, 6], FP32, kind="ExternalInput")
    # xd row p: chunks 1..8, per chunk [i=0..1][c] = x[i*128+p, s+c]
    xd_d = nc.dram_tensor("xd", [128, 7 * 1024], BF16, kind="ExternalInput")
    # od row p: per chunk at pixel s width P: cols [2s + o*P + k]
    od_d = nc.dram_tensor("od", [128, 8192], BF16, kind="ExternalOutput")

    Exp = mybir.ActivationFunctionType.Exp
    Ident = mybir.ActivationFunctionType.Identity
    fused = _register_fused_divmul()

    # xd column offset of chunk j (j >= 1); chunk 0's x lives in b1/b2
    xoff = [0] * NCH
    for j in range(1, NCH):
        xoff[j] = 2 * (STARTS[j] - 512)

    with tile.TileContext(nc) as tc:
        with (
            tc.tile_pool(name="wpool", bufs=1) as wpool,
            tc.tile_pool(name="warm", bufs=1) as warm,
            tc.tile_pool(name="xpool", bufs=4) as xpool,
            tc.tile_pool(name="epool", bufs=5) as epool,
            tc.tile_pool(name="spool", bufs=6) as spool,
            tc.tile_pool(name="opool", bufs=3) as opool,
            tc.tile_pool(name="pa", bufs=4, space="PSUM") as pa,
            tc.tile_pool(name="ps", bufs=2, space="PSUM") as ps,
            tc.tile_pool(name="po", bufs=1, space="PSUM") as po,
        ):
            # --- act-table preload: dummy exp (gated only on the wz
            # memset) is the FIRST scalar instruction, so the lazy
            # ACT_TABLE_LOAD runs in the DMA-free window before b1's
            # packets start, and the first real exp is then gated only
            # by its PSUM input.
            wz = warm.tile([128, 256], BF16, name="wz", tag="wz")
            nc.gpsimd.memset(wz[:], 0)
            ez = warm.tile([128, 16], BF16, name="ez", tag="ez")
            nc.scalar.activation(ez[:], wz[:, 0:16], Exp)

            # --- DMAs, latency-ordered. b1+b2 serial on the sync queue
            # (b1 gets full DMA throughput); x1 on the scalar queue in
            # two halves so chunk 1's i=0 matmuls can start early.
            b1 = wpool.tile([128, 1024], BF16, name="b1", tag="b1")
            nc.sync.dma_start(b1[:], b1_d[:, :])
            b2 = wpool.tile([128, 1024], BF16, name="b2", tag="b2")
            nc.sync.dma_start(b2[:], b2_d[:, :])
            wrest = wpool.tile([128, 1152], BF16, name="wrest", tag="wrest")
            nc.sync.dma_start(wrest[:], wrest_d[:, :])
            wf = wpool.tile([128, 6], FP32, name="wf", tag="wf")
            nc.sync.dma_start(wf[:], wf_d[:, :])

            xt = {}  # chunk j -> (tile, col offset)

            def load_x(j, eng=None, halves=False):
                t_ = xpool.tile([128, 1024], BF16, name=f"x{j}", tag="x")
                q = eng or nc.sync
                w = 2 * CHUNKS[j] + (2 * CHUNKS[j + 1] if j == NCH - 2 else 0)
                if halves:
                    q.dma_start(t_[:, 0 : w // 2],
                                xd_d[:, xoff[j] : xoff[j] + w // 2])
                    q.dma_start(t_[:, w // 2 : w],
                                xd_d[:, xoff[j] + w // 2 : xoff[j] + w])
                else:
                    q.dma_start(t_[:, 0:w], xd_d[:, xoff[j] : xoff[j] + w])
                xt[j] = (t_, 0)
                if j == NCH - 2:
                    xt[j + 1] = (t_, 2 * CHUNKS[j])

            load_x(1, nc.scalar, halves=True)
            load_x(2)
            load_x(3)

            # --- PE warmup: ramp the p-state from the very start of the
            # instruction stream; gated only on the wz memset.
            for w in range(16):
                wp = pa.tile([128, 512], FP32, name=f"warm{w}", tag="pa")
                fz = 128 if w < 10 else 256
                nc.tensor.matmul(
                    wp[:, 0:fz], wz[:, 0:128], wz[:, 0:fz],
                    start=True, stop=True,
                )

            kw = [b1[:, 0:512], b2[:, 0:512]]
            sumw = wrest[:, 0:128]

            prev = None  # (s_tiles, po_t, j_prev)

            def issue_B(state):
                s_tiles, po_t, jp = state
                P = CHUNKS[jp]
                for t in range(4):
                    for o in range(2):
                        nc.tensor.matmul(
                            po_t[o][:, 0:P],
                            wrest[:, 128 + t * 256 + o * 128 : 128 + t * 256 + (o + 1) * 128],
                            s_tiles[t][:, 0:P],
                            start=(t == 0),
                            stop=(t == 3),
                        )

            def issue_tail(state, last=False):
                s_tiles, po_t, jp = state
                P = CHUNKS[jp]
                s0 = 2 * STARTS[jp]
                o_sb = opool.tile([128, 1024], BF16, name=f"o_{jp}", tag="o")
                nc.scalar.activation(
                    o_sb[:, 0:P], po_t[0][:, 0:P], Ident, bias=wf[:, 4:5]
                )
                if last:
                    # o0 half goes out via the scalar engine's own HWDGE
                    # queue, in parallel with sync handling the o1 half.
                    nc.scalar.dma_start(
                        od_d[:, s0 : s0 + P], o_sb[:, 0:P]
                    )
                nc.vector.tensor_scalar_add(
                    o_sb[:, P : 2 * P], po_t[1][:, 0:P], wf[:, 5:6]
                )
                if last:
                    nc.sync.dma_start(
                        od_d[:, s0 + P : s0 + 2 * P], o_sb[:, P : 2 * P]
                    )
                else:
                    nc.sync.dma_start(
                        od_d[:, s0 : s0 + 2 * P], o_sb[:, 0 : 2 * P]
                    )

            for j in range(NCH):
                P = CHUNKS[j]
                if j == 0:
                    xi = [b1[:, 512:1024], b2[:, 512:1024]]
                else:
                    xc, off = xt.pop(j)
                    xi = [xc[:, off : off + P], xc[:, off + P : off + 2 * P]]

                # ---- stage A ----
                a_ps = [
                    pa.tile([128, 512], FP32, name=f"pa_{j}_{t}", tag="pa")
                    for t in range(4)
                ]
                if j <= 1:
                    # i-half-major: the 4 i=0 matmuls only need the first
                    # half of the chunk's input (blob1 / x1's first half).
                    for i in range(2):
                        for t in range(4):
                            nc.tensor.matmul(
                                a_ps[t][:, 0:P],
                                kw[i][:, t * 128 : (t + 1) * 128],
                                xi[i],
                                start=(i == 0),
                                stop=(i == 1),
                            )
                else:
                    for t in range(4):
                        for i in range(2):
                            nc.tensor.matmul(
                                a_ps[t][:, 0:P],
                                kw[i][:, t * 128 : (t + 1) * 128],
                                xi[i],
                                start=(i == 0),
                                stop=(i == 1),
                            )

                e_sb = []
                for t in range(4):
                    e_ = epool.tile([128, 512], BF16, name=f"e_{j}_{t}", tag="e")
                    nc.scalar.activation(
                        e_[:, 0:P], a_ps[t][:, 0:P], Exp, bias=wf[:, t : t + 1]
                    )
                    e_sb.append(e_)

                s_tiles = []
                for t in range(2):
                    p_ = ps.tile([128, 512], FP32, name=f"ps_{j}_{t}", tag="ps")
                    nc.tensor.matmul(
                        p_[:, 0:P], sumw, e_sb[t][:, 0:P], start=True, stop=True
                    )
                    s_ = spool.tile([128, 512], BF16, name=f"s_{j}_{t}", tag="s")
                    nc.vector._custom_dve(
                        fused, out=s_[:, 0:P], in0=p_[:, 0:P], in1=e_sb[t][:, 0:P],
                        s0=_RC0, s1=_RC1,
                    )
                    s_tiles.append(s_)

                if prev is not None:
                    issue_B(prev)

                for t in range(2, 4):
                    p_ = ps.tile([128, 512], FP32, name=f"ps_{j}_{t}", tag="ps")
                    nc.tensor.matmul(
                        p_[:, 0:P], sumw, e_sb[t][:, 0:P], start=True, stop=True
                    )
                    s_ = spool.tile([128, 512], BF16, name=f"s_{j}_{t}", tag="s")
                    nc.vector._custom_dve(
                        fused, out=s_[:, 0:P], in0=p_[:, 0:P], in1=e_sb[t][:, 0:P],
                        s0=_RC0, s1=_RC1,
                    )
                    s_tiles.append(s_)

                if prev is not None:
                    issue_tail(prev)

                po_t = [
                    po.tile([128, 512], FP32, name=f"po{o}_{j}", tag=f"po{o}")
                    for o in range(2)
                ]
                prev = (s_tiles, po_t, j)

                if j + 4 <= NCH - 2:
                    load_x(j + 4)

            issue_B(prev)
            issue_tail(prev, last=True)

    nc.compile()
    return nc


def _fold_weights(key_p, memory, w_in, b_in, w_out, b_out):
    import ml_dtypes

    key_p = np.asarray(key_p, np.float64)
    memory = np.asarray(memory, np.float64)
    w_in = np.asarray(w_in, np.float64)
    b_in = np.asarray(b_in, np.float64)
    w_out = np.asarray(w_out, np.float64)
    b_out = np.asarray(b_out, np.float64)

    w_in_r = w_in.reshape(64, 8, C_IN)  # [k, n, c]
    kw = np.einsum("nkm,knc->nmc", key_p, w_in_r)  # [n, m, c]
    kwt = kw.reshape(NM, C_IN).T.astype(ml_dtypes.bfloat16)  # [c, nm]

    kb = np.einsum("nkm,kn->nm", key_p, b_in.reshape(64, 8)).reshape(NM)

    w_out_r = w_out.reshape(C_OUT, 8, 64)  # [o, n, d]
    wm = np.einsum("ond,nmd->onm", w_out_r, memory)  # [o, n, m]
    wmt = wm.reshape(C_OUT, NM).T  # [nm, o]

    wrest = np.zeros((128, 1152), ml_dtypes.bfloat16)
    blk = np.zeros((128, 128))
    blk[:64, :64] = 1.0
    blk[64:, 64:] = 1.0
    wrest[:, 0:128] = blk.astype(ml_dtypes.bfloat16)
    for t in range(4):
        wrest[:, 128 + t * 256 : 128 + (t + 1) * 256] = (
            wmt[t * 128 : (t + 1) * 128, :].astype(ml_dtypes.bfloat16)
        )

    wf = np.zeros((128, 6), np.float32)
    wf[:, 0:4] = kb.reshape(4, 128).T
    wf[:, 4:6] = b_out.reshape(2, 128).T
    return kwt, np.ascontiguousarray(wrest), wf


import ml_dtypes as _mld

_ml_bf16 = _mld.bfloat16


def kernel_with_results(trace=False, tmpdir=None, **inputs):
    global _CACHED_NC
    x = np.asarray(inputs["x"], np.float32)  # [8, 256, 64, 64]
    kwt, wrest, wf = _fold_weights(
        inputs["key_p"],
        inputs["memory"],
        inputs["w_in"],
        inputs["b_in"],
        inputs["w_out"],
        inputs["b_out"],
    )
    if _CACHED_NC is None:
        _CACHED_NC = _build_nc()
    nc = _CACHED_NC

    in_maps = []
    for b in range(N_CORES):
        xb = x[b].reshape(C_IN, NPIX).astype(_ml_bf16)  # [c, pix]
        b1 = np.concatenate([kwt[0:128], xb[0:128, 0:512]], axis=1)
        b2 = np.concatenate([kwt[128:256], xb[128:256, 0:512]], axis=1)
        # xd: chunks 1..8, per chunk [i0 block | i1 block]
        cols = []
        for j in range(1, NCH):
            s, P = STARTS[j], CHUNKS[j]
            cols.append(xb[0:128, s : s + P])
            cols.append(xb[128:256, s : s + P])
        xd = np.concatenate(cols, axis=1)
        in_maps.append(
            {
                "b1": np.ascontiguousarray(b1),
                "b2": np.ascontiguousarray(b2),
                "wrest": wrest,
                "wf": wf,
                "xd": np.ascontiguousarray(xd),
            }
        )

    try:
        res = bass_utils.run_bass_kernel_spmd(
            nc, in_maps, core_ids=list(range(N_CORES)), trace=trace, tmpdir=tmpdir
        )
    except Exception:
        # transient NRT/device hiccups (e.g. a wedged core from a prior
        # tenant) usually clear on a retry
        res = bass_utils.run_bass_kernel_spmd(
            nc, in_maps, core_ids=list(range(N_CORES)), trace=trace, tmpdir=tmpdir
        )
    outs = []
    for b in range(N_CORES):
        od = np.asarray(res.results[b]["od"]).astype(np.float32)
        out = np.empty((C_OUT, NPIX), np.float32)
        for j in range(NCH):
            s, P = STARTS[j], CHUNKS[j]
            blk = od[:, 2 * s : 2 * s + 2 * P]
            out[0:128, s : s + P] = blk[:, 0:P]
            out[128:256, s : s + P] = blk[:, P : 2 * P]
        outs.append(out.reshape(C_OUT, 64, 64))
    out = np.stack(outs)
    return out, res


def kernel(**inputs):
    out, _ = kernel_with_results(trace=False, **inputs)
    return out


# revision 20
# speedup vs baseline: 1.0376x; 1.0006x over previous
"""Trainium2 Bass kernel for KeyChannelwiseMemoryMultiHead.

Math per pixel vector x (256 channels):
  y1 = w_in @ x + b_in; per-head key matmul; softmax over mem dim;
  per-head memory matmul; w_out @ . + b_out.

Host-side exact refactor (fp64 weight folding):
  KW[(n,m), c] = sum_k key_p[n,k,m] w_in[k*8+n, c]   -> stage A: A = KW @ x
  kb[(n,m)]    = sum_k key_p[n,k,m] b_in[k*8+n]      -> folded as exp bias
  WM[o, (n,m)] = sum_d w_out[o, n*64+d] memory[n,m,d]
  E = exp(A + kb);  wsum[n] = sum_m E;  S = E / wsum
  out = WM @ S + b_out

On-chip (per core = one batch, pixel chunks [512 x7, 256, 256]):
  stage A: 2 K-tile bf16 matmuls -> PSUM [128 nm, P pix] (4 nm tiles)
  exp:     ScalarE activation(Exp, bias=kb) PSUM->SBUF
  wsum:    matmul with block-diagonal ones [128,128] (head-indicator)
  recip:   fused DVE op S = E * approx_recip(wsum)  PSUM->SBUF
  stage B: 8 accumulating bf16 matmuls -> PSUM [128 out, P pix]
  bias:    ScalarE identity+bias (o=0) / DVE tensor_scalar_add (o=1), bf16.

Perf structure:
  - The head is HBM-bandwidth-bound (8 cores share the fabric; per-core
    ~125GB/s effective): kw + chunk-0 x are fused into 2 blobs serial on
    the sync queue, chunk 0 runs i-half-major so blob1 alone unblocks 4
    matmuls. Chunk 1's x rides the scalar queue as TWO half DMAs so its
    i=0 matmuls can start as soon as the first half lands.
  - A dummy exp (gated only on the wz memset) heads the scalar stream so
    the lazy ACT_TABLE_LOAD's ~1.3us table fetch runs in the DMA-free
    window before b1's packets start.
  - 16 small warmup matmuls (gated only on the wz memset) start the PE
    p-state ramp at ~6.1us, right when the instruction stream begins
    (PE clock ramps 0.65->2.4GHz over ~3-4.5us of continuous activity;
    idle gaps >~0.5us reset the ramp), so the first real A matmuls and
    the whole first chunk run near full clock.
  - The last two chunks are 256px: the final drain (exp/recip already
    overlapped; B + bias + last output DMA remain) covers half the
    bytes, and the final DMA is split o0/o1 across scalar/sync queues.
  - Tensor-engine issue order per chunk j: A(j) | wsum(j,t0,t1) | B(j-1) |
    wsum(j,t2,t3) -- hides exp/recip latency behind the previous chunk's B.
  - PSUM banks: pa=4, ps=2, po=2x1.
  - Output stored bf16 (host converts to fp32).
"""

import os
import sys

import numpy as np

for _p in ("/opt/trn_rl_repo", "/root/.axon_site/_ro/trn_rl_repo"):
    if os.path.isdir(_p) and _p not in sys.path:
        sys.path.insert(0, _p)

import concourse.bass as bass  # noqa: E402
import concourse.tile as tile  # noqa: E402
from concourse import bacc, bass_utils, mybir  # noqa: E402
from concourse import dve_ops as _dve_ops  # noqa: E402
from concourse.dve_spec import (  # noqa: E402
    AluOp,
    Bin,
    C0,
    C1,
    Spec,
    Src0,
    Src1,
    _has_src1,
    lower,
)
from concourse.dve_uop import DveOpSpec  # noqa: E402

N_CORES = 8
C_IN = 256
NM = 512
C_OUT = 256
NPIX = 64 * 64
FP32 = mybir.dt.float32
BF16 = mybir.dt.bfloat16
_RC0 = -0.23549792
_RC1 = 2.0017324

CHUNKS = [512, 512, 512, 512, 512, 512, 512, 256, 256]
STARTS = [sum(CHUNKS[:i]) for i in range(len(CHUNKS))]
NCH = len(CHUNKS)
assert sum(CHUNKS) == NPIX

_FUSED_OP = None


def _register_fused_divmul():
    """out = in1 * approx_recip(in0): BITWISE_NOT exponent-flip seed +
    one inline Newton pass + multiply by in1 -- single DVE pass."""
    global _FUSED_OP
    if _FUSED_OP is not None:
        return _FUSED_OP
    name = "RECIP1NR_MUL_ANT"
    _not_x = Bin(AluOp.BITWISE_NOT, Src0, Src0)
    _y0 = _not_x * C0
    _y1 = _y0 * (C1 - Src0 * _y0)

    def _ref(in0, in1, c0, c1, c2):
        not_x = (~in0.view(np.int32)).view(np.float32)
        y0 = not_x * c0
        y1 = y0 * (c1 - in0 * y0)
        return y1 * in1

    spec = Spec(body=_y1 * Src1, reference=_ref)
    row = max(_dve_ops._SUB_OPCODE_FOR_NAME.values()) + 1
    assert row < 0x20
    _dve_ops._SUB_OPCODE_FOR_NAME[name] = row
    shas = {}
    for ver in ("v3",):
        s = DveOpSpec(name=name, opcode=row, uops=lower(spec, ver=ver),
                      rd1_en=_has_src1(spec))
        shas[ver] = s.sha(ver)
    op = _dve_ops.DveOp(name, spec, subdim=False, uops_sha=shas)
    _dve_ops.OPS.append(op)
    _dve_ops.CUSTOM_DVE_SPECS[name] = spec
    _FUSED_OP = op
    return op

_CACHED_NC = None


def _build_nc():
    nc = bacc.Bacc(
        "TRN2",
        target_bir_lowering=False,
        debug=False,
        enable_asserts=True,
        num_devices=N_CORES,
    )
    # b1 row p: [kwt[p, :] (512) | x[p, 0:512]]      (k-half 0 weights + x)
    # b2 row p: [kwt[128+p, :] (512) | x[128+p, 0:512]]
    b1_d = nc.dram_tensor("b1", [128, 1024], BF16, kind="ExternalInput")
    b2_d = nc.dram_tensor("b2", [128, 1024], BF16, kind="ExternalInput")
    # wrest cols: [sumw 128 | wmt 4x256]
    wrest_d = nc.dram_tensor("wrest", [128, 1152], BF16, kind="ExternalInput")
    # wf cols: [kb tile0..3 | bout o0 | bout o1]
    wf_d = nc.dram_tensor("wf", [128, 6], FP32, kind="ExternalInput")
    # xd row p: chunks 1..8, per chunk [i=0..1][c] = x[i*128+p, s+c]
    xd_d = nc.dram_tensor("xd", [128, 7 * 1024], BF16, kind="ExternalInput")
    # od row p: per chunk at pixel s width P: cols [2s + o*P + k]
    od_d = nc.dram_tensor("od", [128, 8192], BF16, kind="ExternalOutput")

    Exp = mybir.ActivationFunctionType.Exp
    Ident = mybir.ActivationFunctionType.Identity
    fused = _register_fused_divmul()

    # xd column offset of chunk j (j >= 1); chunk 0's x lives in b1/b2
    xoff = [0] * NCH
    for j in range(1, NCH):
        xoff[j] = 2 * (STARTS[j] - 512)

    with tile.TileContext(nc) as tc:
        with (
            tc.tile_pool(name="wpool", bufs=1) as wpool,
            tc.tile_pool(name="warm", bufs=1) as warm,
            tc.tile_pool(name="xpool", bufs=4) as xpool,
            tc.tile_pool(name="epool", bufs=5) as epool,
            tc.tile_pool(name="spool", bufs=6) as spool,
            tc.tile_pool(name="opool", bufs=3) as opool,
            tc.tile_pool(name="pa", bufs=4, space="PSUM") as pa,
            tc.tile_pool(name="ps", bufs=2, space="PSUM") as ps,
            tc.tile_pool(name="po", bufs=1, space="PSUM") as po,
        ):
            # --- act-table preload: dummy exp (gated only on the wz
            # memset) is the FIRST scalar instruction, so the lazy
            # ACT_TABLE_LOAD runs in the DMA-free window before b1's
            # packets start, and the first real exp is then gated only
            # by its PSUM input.
            wz = warm.tile([128, 256], BF16, name="wz", tag="wz")
            nc.gpsimd.memset(wz[:], 0)
            ez = warm.tile([128, 16], BF16, name="ez", tag="ez")
            nc.scalar.activation(ez[:], wz[:, 0:16], Exp)

            # --- DMAs, latency-ordered. b1+b2 serial on the sync queue
            # (b1 gets full DMA throughput); x1 on the scalar queue in
            # two halves so chunk 1's i=0 matmuls can start early.
            b1 = wpool.tile([128, 1024], BF16, name="b1", tag="b1")
            nc.sync.dma_start(b1[:], b1_d[:, :])
            b2 = wpool.tile([128, 1024], BF16, name="b2", tag="b2")
            nc.sync.dma_start(b2[:], b2_d[:, :])
            wrest = wpool.tile([128, 1152], BF16, name="wrest", tag="wrest")
            nc.sync.dma_start(wrest[:], wrest_d[:, :])
            wf = wpool.tile([128, 6], FP32, name="wf", tag="wf")
            nc.sync.dma_start(wf[:], wf_d[:, :])

            xt = {}  # chunk j -> (tile, col offset)

            def load_x(j, eng=None, halves=False):
                t_ = xpool.tile([128, 1024], BF16, name=f"x{j}", tag="x")
                q = eng or nc.sync
                w = 2 * CHUNKS[j] + (2 * CHUNKS[j + 1] if j == NCH - 2 else 0)
                if halves:
                    q.dma_start(t_[:, 0 : w // 2],
                                xd_d[:, xoff[j] : xoff[j] + w // 2])
                    q.dma_start(t_[:, w // 2 : w],
                                xd_d[:, xoff[j] + w // 2 : xoff[j] + w])
                else:
                    q.dma_start(t_[:, 0:w], xd_d[:, xoff[j] : xoff[j] + w])
                xt[j] = (t_, 0)
                if j == NCH - 2:
                    xt[j + 1] = (t_, 2 * CHUNKS[j])

            load_x(1, nc.scalar, halves=True)
            load_x(2)
            load_x(3)

            # --- PE warmup: ramp the p-state from the very start of the
            # instruction stream; gated only on the wz memset.
            # sizes taper down so that when b1's semaphore fires, the
            # warmups still queued ahead of the first real ldweights
            # flush in well under a microsecond.
            WARM_F = (256, 256, 256, 256, 256, 256, 128, 128, 128, 128,
                      64, 64, 64, 32, 16, 16)
            for w, fz in enumerate(WARM_F):
                wp = pa.tile([128, 512], FP32, name=f"warm{w}", tag="pa")
                nc.tensor.matmul(
                    wp[:, 0:fz], wz[:, 0:128], wz[:, 0:fz],
                    start=True, stop=True,
                )

            kw = [b1[:, 0:512], b2[:, 0:512]]
            sumw = wrest[:, 0:128]

            prev = None  # (s_tiles, po_t, j_prev)

            def issue_B(state):
                s_tiles, po_t, jp = state
                P = CHUNKS[jp]
                for t in range(4):
                    for o in range(2):
                        nc.tensor.matmul(
                            po_t[o][:, 0:P],
                            wrest[:, 128 + t * 256 + o * 128 : 128 + t * 256 + (o + 1) * 128],
                            s_tiles[t][:, 0:P],
                            start=(t == 0),
                            stop=(t == 3),
                        )

            def issue_tail(state, last=False):
                s_tiles, po_t, jp = state
                P = CHUNKS[jp]
                s0 = 2 * STARTS[jp]
                o_sb = opool.tile([128, 1024], BF16, name=f"o_{jp}", tag="o")
                nc.scalar.activation(
                    o_sb[:, 0:P], po_t[0][:, 0:P], Ident, bias=wf[:, 4:5]
                )
                if last:
                    # o0 half goes out via the scalar engine's own HWDGE
                    # queue, in parallel with sync handling the o1 half.
                    nc.scalar.dma_start(
                        od_d[:, s0 : s0 + P], o_sb[:, 0:P]
                    )
                nc.vector.tensor_scalar_add(
                    o_sb[:, P : 2 * P], po_t[1][:, 0:P], wf[:, 5:6]
                )
                if last:
                    nc.sync.dma_start(
                        od_d[:, s0 + P : s0 + 2 * P], o_sb[:, P : 2 * P]
                    )
                else:
                    nc.sync.dma_start(
                        od_d[:, s0 : s0 + 2 * P], o_sb[:, 0 : 2 * P]
                    )

            for j in range(NCH):
                P = CHUNKS[j]
                if j == 0:
                    xi = [b1[:, 512:1024], b2[:, 512:1024]]
                else:
                    xc, off = xt.pop(j)
                    xi = [xc[:, off : off + P], xc[:, off + P : off + 2 * P]]

                # ---- stage A ----
                a_ps = [
                    pa.tile([128, 512], FP32, name=f"pa_{j}_{t}", tag="pa")
                    for t in range(4)
                ]
                if j <= 1:
                    # i-half-major: the 4 i=0 matmuls only need the first
                    # half of the chunk's input (blob1 / x1's first half).
                    for i in range(2):
                        for t in range(4):
                            nc.tensor.matmul(
                                a_ps[t][:, 0:P],
                                kw[i][:, t * 128 : (t + 1) * 128],
                                xi[i],
                                start=(i == 0),
                                stop=(i == 1),
                            )
                else:
                    for t in range(4):
                        for i in range(2):
                            nc.tensor.matmul(
                                a_ps[t][:, 0:P],
                                kw[i][:, t * 128 : (t + 1) * 128],
                                xi[i],
                                start=(i == 0),
                                stop=(i == 1),
                            )

                e_sb = []
                for t in range(4):
                    e_ = epool.tile([128, 512], BF16, name=f"e_{j}_{t}", tag="e")
                    nc.scalar.activation(
                        e_[:, 0:P], a_ps[t][:, 0:P], Exp, bias=wf[:, t : t + 1]
                    )
                    e_sb.append(e_)

                s_tiles = []
                for t in range(2):
                    p_ = ps.tile([128, 512], FP32, name=f"ps_{j}_{t}", tag="ps")
                    nc.tensor.matmul(
                        p_[:, 0:P], sumw, e_sb[t][:, 0:P], start=True, stop=True
                    )
                    s_ = spool.tile([128, 512], BF16, name=f"s_{j}_{t}", tag="s")
                    nc.vector._custom_dve(
                        fused, out=s_[:, 0:P], in0=p_[:, 0:P], in1=e_sb[t][:, 0:P],
                        s0=_RC0, s1=_RC1,
                    )
                    s_tiles.append(s_)

                if prev is not None:
                    issue_B(prev)

                for t in range(2, 4):
                    p_ = ps.tile([128, 512], FP32, name=f"ps_{j}_{t}", tag="ps")
                    nc.tensor.matmul(
                        p_[:, 0:P], sumw, e_sb[t][:, 0:P], start=True, stop=True
                    )
                    s_ = spool.tile([128, 512], BF16, name=f"s_{j}_{t}", tag="s")
                    nc.vector._custom_dve(
                        fused, out=s_[:, 0:P], in0=p_[:, 0:P], in1=e_sb[t][:, 0:P],
                        s0=_RC0, s1=_RC1,
                    )
                    s_tiles.append(s_)

                if prev is not None:
                    issue_tail(prev)

                po_t = [
                    po.tile([128, 512], FP32, name=f"po{o}_{j}", tag=f"po{o}")
                    for o in range(2)
                ]
                prev = (s_tiles, po_t, j)

                if j + 4 <= NCH - 2:
                    load_x(j + 4)

            issue_B(prev)
            issue_tail(prev, last=True)

    nc.compile()
    return nc


def _fold_weights(key_p, memory, w_in, b_in, w_out, b_out):
    import ml_dtypes

    key_p = np.asarray(key_p, np.float64)
    memory = np.asarray(memory, np.float64)
    w_in = np.asarray(w_in, np.float64)
    b_in = np.asarray(b_in, np.float64)
    w_out = np.asarray(w_out, np.float64)
    b_out = np.asarray(b_out, np.float64)

    w_in_r = w_in.reshape(64, 8, C_IN)  # [k, n, c]
    kw = np.einsum("nkm,knc->nmc", key_p, w_in_r)  # [n, m, c]
    kwt = kw.reshape(NM, C_IN).T.astype(ml_dtypes.bfloat16)  # [c, nm]

    kb = np.einsum("nkm,kn->nm", key_p, b_in.reshape(64, 8)).reshape(NM)

    w_out_r = w_out.reshape(C_OUT, 8, 64)  # [o, n, d]
    wm = np.einsum("ond,nmd->onm", w_out_r, memory)  # [o, n, m]
    wmt = wm.reshape(C_OUT, NM).T  # [nm, o]

    wrest = np.zeros((128, 1152), ml_dtypes.bfloat16)
    blk = np.zeros((128, 128))
    blk[:64, :64] = 1.0
    blk[64:, 64:] = 1.0
    wrest[:, 0:128] = blk.astype(ml_dtypes.bfloat16)
    for t in range(4):
        wrest[:, 128 + t * 256 : 128 + (t + 1) * 256] = (
            wmt[t * 128 : (t + 1) * 128, :].astype(ml_dtypes.bfloat16)
        )

    wf = np.zeros((128, 6), np.float32)
    wf[:, 0:4] = kb.reshape(4, 128).T
    wf[:, 4:6] = b_out.reshape(2, 128).T
    return kwt, np.ascontiguousarray(wrest), wf


import ml_dtypes as _mld

_ml_bf16 = _mld.bfloat16


def kernel_with_results(trace=False, tmpdir=None, **inputs):
    global _CACHED_NC
    x = np.asarray(inputs["x"], np.float32)  # [8, 256, 64, 64]
    kwt, wrest, wf = _fold_weights(
        inputs["key_p"],
        inputs["memory"],
        inputs["w_in"],
        inputs["b_in"],
        inputs["w_out"],
        inputs["b_out"],
    )
    if _CACHED_NC is None:
        _CACHED_NC = _build_nc()
    nc = _CACHED_NC

    in_maps = []
    for b in range(N_CORES):
        xb = x[b].reshape(C_IN, NPIX).astype(_ml_bf16)  # [c, pix]
        b1 = np.concatenate([kwt[0:128], xb[0:128, 0:512]], axis=1)
        b2 = np.concatenate([kwt[128:256], xb[128:256, 0:512]], axis=1)
        # xd: chunks 1..8, per chunk [i0 block | i1 block]
        cols = []
        for j in range(1, NCH):
            s, P = STARTS[j], CHUNKS[j]
            cols.append(xb[0:128, s : s + P])
            cols.append(xb[128:256, s : s + P])
        xd = np.concatenate(cols, axis=1)
        in_maps.append(
            {
                "b1": np.ascontiguousarray(b1),
                "b2": np.ascontiguousarray(b2),
                "wrest": wrest,
                "wf": wf,
                "xd": np.ascontiguousarray(xd),
            }
        )

    try:
        res = bass_utils.run_bass_kernel_spmd(
            nc, in_maps, core_ids=list(range(N_CORES)), trace=trace, tmpdir=tmpdir
        )
    except Exception:
        # transient NRT/device hiccups (e.g. a wedged core from a prior
        # tenant) usually clear on a retry
        res = bass_utils.run_bass_kernel_spmd(
            nc, in_maps, core_ids=list(range(N_CORES)), trace=trace, tmpdir=tmpdir
        )
    outs = []
    for b in range(N_CORES):
        od = np.asarray(res.results[b]["od"]).astype(np.float32)
        out = np.empty((C_OUT, NPIX), np.float32)
        for j in range(NCH):
            s, P = STARTS[j], CHUNKS[j]
            blk = od[:, 2 * s : 2 * s + 2 * P]
            out[0:128, s : s + P] = blk[:, 0:P]
            out[128:256, s : s + P] = blk[:, P : 2 * P]
        outs.append(out.reshape(C_OUT, 64, 64))
    out = np.stack(outs)
    return out, res


def kernel(**inputs):
    out, _ = kernel_with_results(trace=False, **inputs)
    return out
